# revision 1
# baseline (speedup 1.0000x reference)
"""2-layer GAT (GATConv + elu, masked output) on 8 Trainium2 NeuronCores.

Sharding: nodes partitioned by contiguous range across 8 cores; each edge is
owned by the core owning its dst. Per layer a DRAM "feature table" holds rows
[hx | 1.0 | alpha_s | alpha_d] so one dma_gather by src fetches all src-side
data per edge. Segment softmax skips segment-max (shift-invariant, alphas are
O(+-5)) and folds normalization: U|D = sum_e ex*[hx|1] via one-hot matmuls
accumulated in PSUM per 128-node block, out = U/D. Layer-2 table is built
from the own h1 shard and AllGather'ed across cores.

kernel(**inputs) takes the FULL reference inputs and returns h2[mask].
"""

import sys
from dataclasses import dataclass

import numpy as np

sys.path.insert(0, "/opt/trn_rl_repo")

PAD_DSTNB = 999.0  # S-build sentinel for pad edges (any value >= 128)


@dataclass(frozen=True)
class Cfg:
    N: int = 100000       # nodes
    E: int = 1600000      # edges (before self loops)
    F: int = 64           # input features
    C: int = 64           # layer-1 out features (H*C)
    O: int = 32           # layer-2 out features (H*O)
    ED: int = 16          # edge-attr dim
    NCORES: int = 8
    CHUNK: int = 32768    # gather row range per chunk (int16 limit)
    WNB: int = 14         # node blocks per window
    PIECE_COLS: int = 8   # max 128-edge blocks per gather piece

    @property
    def NPC(self):  # real nodes per core
        assert self.N % self.NCORES == 0
        return self.N // self.NCORES

    @property
    def NB(self):   # node blocks per core
        return (self.NPC + 127) // 128

    @property
    def NPCP(self):  # padded nodes per core
        return self.NB * 128

    @property
    def NTOT(self):  # padded node space
        return self.NCORES * self.NPCP

    @property
    def NCHUNK(self):
        return (self.NTOT + self.CHUNK - 1) // self.CHUNK

    @property
    def NWIN(self):
        assert self.NB % self.WNB == 0
        return self.NB // self.WNB

    @property
    def WIN_NODES(self):
        return self.WNB * 128

    # table row sizes (fp32 elems); rows are [hx | 1.0 | a_s | a_d | pad]
    @property
    def EL1(self):
        need = self.C + 3
        return 128 if need > 64 else 64

    @property
    def EL2(self):
        need = self.O + 3
        return 128 if need > 64 else 64


CFG_FULL = Cfg()


def gid(cfg, n):
    return (n // cfg.NPC) * cfg.NPCP + (n % cfg.NPC)


# ===================================================================== host
def make_key(cfg, core, nb_local, chunk):
    """Stream ordering: core-major, then window, then chunk, then nb."""
    c = cfg
    w = nb_local // c.WNB
    nbw = nb_local % c.WNB
    return ((core * c.NWIN + w) * c.NCHUNK + chunk) * c.WNB + nbw


def prepare2(cfg, edge_index, edge_attr):
    """Shard edges; streams are (window, chunk, nb)-ordered so gather
    pieces within one (window, chunk) run are large and single-chunk."""
    c = cfg
    src = edge_index[0].astype(np.int64)
    dst = edge_index[1].astype(np.int64)
    loops = np.arange(c.N, dtype=np.int64)
    src = np.concatenate([src, loops])
    dst = np.concatenate([dst, loops])
    eidx = np.concatenate([np.arange(c.E, dtype=np.int64),
                           np.full(c.N, -1, dtype=np.int64)])

    gsrc = gid(c, src)
    core = dst // c.NPC
    nb_local = (dst % c.NPC) // 128
    dst_nb = (dst % c.NPC) % 128
    chunk = gsrc // c.CHUNK
    src_local = (gsrc % c.CHUNK).astype(np.int64)

    nkey = c.NWIN * c.NCHUNK * c.WNB
    key = make_key(c, core, nb_local, chunk)
    counts = np.bincount(key, minlength=c.NCORES * nkey).reshape(
        c.NCORES, nkey)
    maxc = counts.max(axis=0)                    # [nkey]
    blocks = (maxc + 127) // 128                 # blocks per (w, ch, nbw)
    EP = int(blocks.sum()) * 128

    slot_len = blocks * 128
    starts = np.zeros(nkey, dtype=np.int64)
    starts[1:] = np.cumsum(slot_len)[:-1]

    order = np.argsort(key, kind="stable")
    so_src = src_local[order]
    so_dnb = dst_nb[order]
    so_eid = eidx[order]
    so_key = key[order]
    seg = np.searchsorted(so_key, np.arange(c.NCORES + 1) * nkey)

    streams = []
    for ci in range(c.NCORES):
        lo, hi = seg[ci], seg[ci + 1]
        s_key = so_key[lo:hi] - ci * nkey
        idx16 = np.zeros(EP, np.int16)
        dstnb = np.full(EP, PAD_DSTNB, np.float32)
        eadat = np.zeros((EP, c.ED), np.float32)
        seg_starts = np.searchsorted(s_key, np.arange(nkey))
        rank = np.arange(hi - lo) - seg_starts[s_key]
        pos = starts[s_key] + rank
        idx16[pos] = so_src[lo:hi].astype(np.int16)
        dstnb[pos] = so_dnb[lo:hi].astype(np.float32)
        has = so_eid[lo:hi] >= 0
        eadat[pos[has]] = edge_attr[so_eid[lo:hi][has]]
        streams.append({
            "idx16": np.ascontiguousarray(
                np.tile(idx16.reshape(-1, 16).T, (8, 1))),
            "dstnb": np.ascontiguousarray(dstnb.reshape(-1, 128).T),
            "ea_ct": np.ascontiguousarray(
                eadat.reshape(-1, 128, c.ED).transpose(1, 0, 2)
                .reshape(128, -1)),
        })
    return streams, dict(EP=EP, blocks=blocks.reshape(
        c.NWIN, c.NCHUNK, c.WNB))


def plan(cfg, blocks):
    """-> windows: list over w of dict(pieces=[...], nb_first={}, nb_last={}).

    piece = dict(chunk, col0, ncols, blks=[(col, nbw), ...])
    first/last: per nb (within window) the (col) of its first/last block.
    """
    c = cfg
    windows = []
    col = 0
    for w in range(c.NWIN):
        pieces = []
        for ch in range(c.NCHUNK):
            run = []  # (col, nbw, run_start, run_stop) for this (w, ch)
            for nbw in range(c.WNB):
                k = int(blocks[w, ch, nbw])
                for i in range(k):
                    run.append((col, nbw, i == 0, i == k - 1))
                    col += 1
            # split run into pieces
            for i in range(0, len(run), c.PIECE_COLS):
                sub = run[i:i + c.PIECE_COLS]
                pieces.append(dict(chunk=ch, col0=sub[0][0], ncols=len(sub),
                                   blks=sub))
        windows.append(dict(pieces=pieces))
    return windows


# ===================================================================== bass
def build_program(cfg, meta):
    import concourse.bass as bass
    import concourse.tile as tile
    import concourse.mybir as mybir
    from concourse import bacc
    from contextlib import ExitStack

    c = cfg
    dt = mybir.dt
    AF = mybir.ActivationFunctionType
    ALU = mybir.AluOpType
    AX = mybir.AxisListType
    EP = meta["EP"]
    windows = plan(c, meta["blocks"])
    NTILE = c.NTOT // 128

    nc = bacc.Bacc("TRN2", target_bir_lowering=False, debug=False,
                   num_devices=c.NCORES)

    f32 = dt.float32
    xT = nc.dram_tensor("xT", [c.F, c.NTOT], f32, kind="ExternalInput").ap()
    xTo = nc.dram_tensor("xTown", [c.F, c.NPCP], f32,
                         kind="ExternalInput").ap()
    idx16 = nc.dram_tensor("idx16", [128, EP // 16], dt.int16,
                           kind="ExternalInput").ap()
    dstnb = nc.dram_tensor("dstnb", [128, EP // 128], f32,
                           kind="ExternalInput").ap()
    ea_ct = nc.dram_tensor("ea_ct", [128, (EP // 128) * c.ED], f32,
                           kind="ExternalInput").ap()
    W1 = nc.dram_tensor("W1", [c.F, c.C], f32, kind="ExternalInput").ap()
    W2 = nc.dram_tensor("W2", [c.C, c.O], f32, kind="ExternalInput").ap()
    We1 = nc.dram_tensor("We1", [c.ED, c.C], f32, kind="ExternalInput").ap()
    We2 = nc.dram_tensor("We2", [c.ED, c.O], f32, kind="ExternalInput").ap()
    # P8 rows: 0 a_s1, 1 a_d1, 2 a_e1, 3 a_s2, 4 a_d2, 5 a_e2, 6 b1, 7 b2
    P8 = nc.dram_tensor("P8", [8, 64], f32, kind="ExternalInput").ap()
    h2ownT = nc.dram_tensor("h2ownT", [c.O, c.NPCP], f32,
                            kind="ExternalOutput").ap()

    table1 = nc.dram_tensor("table1", [c.NTOT, c.EL1], f32).ap()
    table2own = nc.dram_tensor("table2own", [c.NPCP, c.EL2], f32).ap()
    table2 = nc.dram_tensor("table2", [c.NTOT, c.EL2], f32).ap()
    h1ownT = nc.dram_tensor("h1ownT", [c.C, c.NPCP], f32,
                            kind="ExternalOutput").ap()
    ad1own = nc.dram_tensor("ad1own", [1, c.NPCP], f32).ap()
    ad2own = nc.dram_tensor("ad2own", [1, c.NPCP], f32).ap()
    ae2d = nc.dram_tensor("ae2d", [128, EP // 128], f32).ap()

    with tile.TileContext(nc) as tc, ExitStack() as ctx:
        consts = ctx.enter_context(tc.tile_pool(name="consts", bufs=1))
        sb = ctx.enter_context(tc.tile_pool(name="sb", bufs=3))
        gp = ctx.enter_context(tc.tile_pool(name="gath", bufs=2))
        pset = ctx.enter_context(tc.tile_pool(name="pset", bufs=1,
                                              space="PSUM"))
        pp = ctx.enter_context(tc.tile_pool(name="ps", bufs=2, space="PSUM"))
        ap_ = ctx.enter_context(tc.tile_pool(name="acc", bufs=1,
                                             space="PSUM"))

        # ---------------- constants / setup
        ident = consts.tile([128, 128], f32, tag="ident")
        ones_t = consts.tile([128, 128], f32, tag="ones")
        nc.vector.memset(ones_t[:], 1.0)
        nc.gpsimd.affine_select(ident[:], ones_t[:], pattern=[[-1, 128]],
                                base=0, channel_multiplier=1,
                                compare_op=ALU.is_equal, fill=0.0)
        iota_i = consts.tile([128, 128], dt.int32, tag="iotai")
        nc.gpsimd.iota(iota_i[:], pattern=[[1, 128]], base=0,
                       channel_multiplier=0)
        iota_f = consts.tile([128, 128], f32, tag="iotaf")
        nc.vector.tensor_copy(iota_f[:], iota_i[:])

        def load_const(ap_dram, p, f_, tag):
            t = consts.tile([p, f_], f32, tag=tag)
            nc.sync.dma_start(t[:], ap_dram)
            return t

        W1_s = load_const(W1[:, :], c.F, c.C, "w1")
        W2_s = load_const(W2[:, :], c.C, c.O, "w2")
        We1_s = load_const(We1[:, :], c.ED, c.C, "we1")
        We2_s = load_const(We2[:, :], c.ED, c.O, "we2")
        P8_s = load_const(P8[:, :], 8, 64, "p8")

        setup_ps = pset.tile([128, 512], f32, tag="setup")

        # packed params -> columns [64, 8]
        nc.tensor.transpose(setup_ps[0:64, 0:8], P8_s[:], ident[0:8, 0:8])
        p8T = consts.tile([64, 8], f32, tag="p8T")
        nc.vector.tensor_copy(p8T[:], setup_ps[0:64, 0:8])
        a_s1c, a_d1c = p8T[:, 0:1], p8T[:, 1:2]
        a_e1c = p8T[:, 2:3]              # [64,1]; only first ED rows matter
        a_s2c, a_d2c = p8T[0:c.O, 3:4], p8T[0:c.O, 4:5]
        a_e2c = p8T[:, 5:6]
        b1c, b2c = p8T[:, 6:7], p8T[0:c.O, 7:8]

        def transp(src, p, f_, tag):
            ps = pp.tile([128, 512], f32, tag="work")
            nc.tensor.transpose(ps[0:f_, 0:p], src[:], ident[0:p, 0:p])
            out = consts.tile([f_, p], f32, tag=tag)
            nc.vector.tensor_copy(out[:], ps[0:f_, 0:p])
            return out

        W1T = transp(W1_s, c.F, c.C, "w1T")     # [C, F]: W1T[o,i]=W1[i,o]
        W2T = transp(W2_s, c.C, c.O, "w2T")     # [O, C]
        We1T = transp(We1_s, c.ED, c.C, "we1T")  # [C, ED]
        We2T = transp(We2_s, c.ED, c.O, "we2T")  # [O, ED]

        # ws/wd columns: ws1[i] = sum_o W1T[o,i]*a_s1[o]
        nc.tensor.matmul(setup_ps[0:c.F, 8:9], W1T[:], a_s1c[0:c.C, :],
                         start=True, stop=True)
        nc.tensor.matmul(setup_ps[0:c.F, 9:10], W1T[:], a_d1c[0:c.C, :],
                         start=True, stop=True)
        nc.tensor.matmul(setup_ps[0:c.C, 10:11], W2T[:], a_s2c, start=True,
                         stop=True)
        nc.tensor.matmul(setup_ps[0:c.C, 11:12], W2T[:], a_d2c, start=True,
                         stop=True)
        wd1c = consts.tile([c.F, 1], f32, tag="wd1c")
        nc.vector.tensor_copy(wd1c[:], setup_ps[0:c.F, 9:10])
        wd2c = consts.tile([c.C, 1], f32, tag="wd2c")
        nc.vector.tensor_copy(wd2c[:], setup_ps[0:c.C, 11:12])

        Wa1 = consts.tile([c.F, c.C + 2], f32, tag="wa1")
        nc.vector.tensor_copy(Wa1[:, 0:c.C], W1_s[:])
        nc.vector.tensor_copy(Wa1[:, c.C:c.C + 2], setup_ps[0:c.F, 8:10])
        Wa2 = consts.tile([c.C, c.O + 2], f32, tag="wa2")
        nc.vector.tensor_copy(Wa2[:, 0:c.O], W2_s[:])
        nc.vector.tensor_copy(Wa2[:, c.O:c.O + 2], setup_ps[0:c.C, 10:12])

        # v rows: v1[d] = sum_c a_e1[c] * We1T[c, d]
        nc.tensor.matmul(setup_ps[0:1, 16:16 + c.ED], a_e1c[0:c.C, :],
                         We1T[:], start=True, stop=True)
        nc.tensor.matmul(setup_ps[0:1, 32:32 + c.ED], a_e2c[0:c.O, :],
                         We2T[:], start=True, stop=True)
        vrow = consts.tile([1, 2 * c.ED], f32, tag="vrow")
        nc.vector.tensor_copy(vrow[:, 0:c.ED], setup_ps[0:1, 16:16 + c.ED])
        nc.vector.tensor_copy(vrow[:, c.ED:], setup_ps[0:1, 32:32 + c.ED])
        V12 = consts.tile([128, 2 * c.ED], f32, tag="V12")
        nc.gpsimd.partition_broadcast(V12[:], vrow[:])
        # replicated [128, PIECE_COLS*ED] patterns
        V1R = consts.tile([128, c.PIECE_COLS * c.ED], f32, tag="V1R")
        V2R = consts.tile([128, c.PIECE_COLS * c.ED], f32, tag="V2R")
        for k in range(c.PIECE_COLS):
            nc.vector.tensor_copy(V1R[:, k * c.ED:(k + 1) * c.ED],
                                  V12[:, 0:c.ED])
            nc.vector.tensor_copy(V2R[:, k * c.ED:(k + 1) * c.ED],
                                  V12[:, c.ED:])

        # ---------------- table build (node-major direct)
        def build_table(inT, ntiles, Wa, hx_w, kin, table, el):
            for t in range(ntiles):
                lhsT = sb.tile([kin, 128], f32, tag="btL")
                nc.sync.dma_start(lhsT[:], inT[:, t * 128:(t + 1) * 128])
                hp = pp.tile([128, 512], f32, tag="work")
                nc.tensor.matmul(hp[:, 0:hx_w + 2], lhsT[:], Wa[:],
                                 start=True, stop=True)
                row = sb.tile([128, hx_w + 3], f32, tag="btR")
                nc.vector.tensor_copy(row[:, 0:hx_w], hp[:, 0:hx_w])
                nc.vector.memset(row[:, hx_w:hx_w + 1], 1.0)
                nc.vector.tensor_copy(row[:, hx_w + 1:hx_w + 3],
                                      hp[:, hx_w:hx_w + 2])
                nc.sync.dma_start(
                    table[t * 128:(t + 1) * 128, 0:hx_w + 3], row[:])

        # ---------------- alpha_d own extraction
        def build_adown(inT, kin, wdcol, adown):
            ad_sb = sb.tile([128, c.NB], f32, tag="adsb")
            for t in range(c.NB):
                lhsT = sb.tile([kin, 128], f32, tag="adL")
                nc.sync.dma_start(lhsT[:], inT[:, t * 128:(t + 1) * 128])
                adp = pp.tile([128, 512], f32, tag="work")
                nc.tensor.matmul(adp[:, 0:1], lhsT[:], wdcol, start=True,
                                 stop=True)
                nc.vector.tensor_copy(ad_sb[:, t:t + 1], adp[:, 0:1])
            adt = pp.tile([128, 512], f32, tag="work")
            nc.tensor.transpose(adt[0:c.NB, 0:128], ad_sb[:], ident[:])
            ad_row = sb.tile([c.NB, 128], f32, tag="adrow")
            nc.vector.tensor_copy(ad_row[:], adt[0:c.NB, 0:128])
            nc.sync.dma_start(
                adown.rearrange("o (a b) -> (o a) b", b=128), ad_row[:])

        # ---------------- edge pass
        def edge_pass(layer, table, el, hx_w, adown, bcol, houtT, hout_w):
            vw = hx_w + 1
            ascol = hx_w + 1  # gathered row col of alpha_s (after the 1.0)
            for w, win in enumerate(windows):
                ad_r = sb.tile([1, c.WIN_NODES], f32, tag="adr")
                nc.sync.dma_start(
                    ad_r[:],
                    adown[:, w * c.WIN_NODES:(w + 1) * c.WIN_NODES])
                AD = sb.tile([128, c.WIN_NODES], f32, tag="AD")
                # chunk the broadcast: long-row pbcast is untrusted on HW
                BC = 256
                for b0 in range(0, c.WIN_NODES, BC):
                    b1 = min(c.WIN_NODES, b0 + BC)
                    nc.gpsimd.partition_broadcast(AD[:, b0:b1],
                                                  ad_r[:, b0:b1])

                # SBUF accumulators per node block; PSUM scratch per
                # (chunk, nb) run -- a run's matmuls are contiguous, so
                # accumulation groups never interleave within a bank.
                accs = [sb.tile([vw, 128], f32, tag=f"acc{k}",
                                name=f"acc{k}_w{w}_l{layer}", bufs=2)
                        for k in range(c.WNB)]
                for a in accs:
                    nc.vector.memset(a[:], 0.0)
                scratch = [None, 0]  # tile, run counter

                for piece in win["pieces"]:
                    ch, col0, ncols = (piece["chunk"], piece["col0"],
                                       piece["ncols"])
                    nidx = ncols * 128
                    rows0 = ch * c.CHUNK
                    rows1 = min(c.NTOT, (ch + 1) * c.CHUNK)
                    # gather
                    it = sb.tile([128, (c.PIECE_COLS * 128) // 16], dt.int16,
                                 tag="idx")
                    nc.sync.dma_start(
                        it[:, 0:nidx // 16],
                        idx16[:, col0 * 8:col0 * 8 + nidx // 16])
                    gt = gp.tile([128, c.PIECE_COLS * el], f32, tag="gt")
                    g3 = gt[:].rearrange("p (n e) -> p n e", e=el)
                    nc.gpsimd.dma_gather(
                        g3[:, 0:ncols, :], table[rows0:rows1, :],
                        it[:, 0:nidx // 16], num_idxs=nidx,
                        num_idxs_reg=nidx, elem_size=el)
                    # streams
                    dnb = sb.tile([128, c.PIECE_COLS], f32, tag="dnb")
                    nc.sync.dma_start(dnb[:, 0:ncols],
                                      dstnb[:, col0:col0 + ncols])
                    aep = sb.tile([128, c.PIECE_COLS], f32, tag="aep")
                    if layer == 1:
                        eat = sb.tile([128, c.PIECE_COLS * c.ED], f32,
                                      tag="eat")
                        nc.sync.dma_start(
                            eat[:, 0:ncols * c.ED],
                            ea_ct[:, col0 * c.ED:(col0 + ncols) * c.ED])
                        tmp = sb.tile([128, c.PIECE_COLS * c.ED], f32,
                                      tag="aetmp")
                        nc.vector.tensor_tensor(
                            tmp[:, 0:ncols * c.ED], eat[:, 0:ncols * c.ED],
                            V1R[:, 0:ncols * c.ED], op=ALU.mult)
                        nc.vector.tensor_reduce(
                            aep[:, 0:ncols],
                            tmp[:].rearrange("p (n e) -> p n e",
                                             e=c.ED)[:, 0:ncols, :],
                            axis=AX.X, op=ALU.add)
                        # alpha_e for layer 2, stored for reuse
                        tmp2 = sb.tile([128, c.PIECE_COLS * c.ED], f32,
                                       tag="aetmp2")
                        nc.vector.tensor_tensor(
                            tmp2[:, 0:ncols * c.ED], eat[:, 0:ncols * c.ED],
                            V2R[:, 0:ncols * c.ED], op=ALU.mult)
                        ae2t = sb.tile([128, c.PIECE_COLS], f32, tag="ae2t")
                        nc.vector.tensor_reduce(
                            ae2t[:, 0:ncols],
                            tmp2[:].rearrange("p (n e) -> p n e",
                                              e=c.ED)[:, 0:ncols, :],
                            axis=AX.X, op=ALU.add)
                        nc.sync.dma_start(ae2d[:, col0:col0 + ncols],
                                          ae2t[:, 0:ncols])
                    else:
                        nc.sync.dma_start(aep[:, 0:ncols],
                                          ae2d[:, col0:col0 + ncols])

                    # per-block S + alpha_d
                    adpe = sb.tile([128, c.PIECE_COLS], f32, tag="adpe")
                    Sp = gp.tile([128, c.PIECE_COLS * 128], f32, tag="Sp")
                    Sts = []
                    for j, (colg, nbw, rs, re) in enumerate(piece["blks"]):
                        S = Sp[:, j * 128:(j + 1) * 128]
                        nc.vector.tensor_scalar(
                            S, iota_f[:], dnb[:, j:j + 1], None,
                            op0=ALU.is_equal)
                        scr = sb.tile([128, 128], f32, tag="scr", bufs=2)
                        nc.vector.scalar_tensor_tensor(
                            scr[:], S, 1.0,
                            AD[:, nbw * 128:(nbw + 1) * 128],
                            op0=ALU.mult, op1=ALU.mult,
                            accum_out=adpe[:, j:j + 1])
                        Sts.append(S)

                    # alpha -> ex (batched over piece)
                    u = sb.tile([128, c.PIECE_COLS], f32, tag="u")
                    nc.vector.tensor_tensor(u[:, 0:ncols],
                                            g3[:, 0:ncols, ascol],
                                            adpe[:, 0:ncols], op=ALU.add)
                    nc.vector.tensor_tensor(u[:, 0:ncols], u[:, 0:ncols],
                                            aep[:, 0:ncols], op=ALU.add)
                    ex = sb.tile([128, c.PIECE_COLS], f32, tag="ex")
                    nc.vector.scalar_tensor_tensor(
                        ex[:, 0:ncols], u[:, 0:ncols], 0.2, u[:, 0:ncols],
                        op0=ALU.mult, op1=ALU.max)
                    nc.scalar.activation(ex[:, 0:ncols], ex[:, 0:ncols],
                                         AF.Exp)

                    # vals + aggregation matmuls (scratch PSUM per run)
                    for j, (colg, nbw, rs, re) in enumerate(piece["blks"]):
                        vals = sb.tile([128, vw], f32, tag=f"v{j % 3}")
                        nc.vector.tensor_scalar(
                            vals[:], g3[:, j, 0:vw], ex[:, j:j + 1], None,
                            op0=ALU.mult)
                        if rs:
                            scratch[0] = ap_.tile(
                                [vw, 128], f32,
                                tag=f"rps{scratch[1] % 3}",
                                name=f"rps_{layer}_{colg}")
                            scratch[1] += 1
                        nc.tensor.matmul(
                            scratch[0][:], vals[:], Sts[j],
                            start=rs, stop=re)
                        if re:
                            nc.vector.tensor_tensor(
                                accs[nbw][:], accs[nbw][:], scratch[0][:],
                                op=ALU.add)

                # finalize nodes of this window
                for nbw in range(c.WNB):
                    acc = accs[nbw]
                    dclamp = sb.tile([1, 128], f32, tag="dcl")
                    nc.vector.tensor_scalar(dclamp[:], acc[vw - 1:vw, :],
                                            1e-30, None, op0=ALU.max)
                    rec = sb.tile([1, 128], f32, tag="rec")
                    nc.vector.reciprocal(rec[:], dclamp[:])
                    RB = sb.tile([hx_w, 128], f32, tag="RB")
                    nc.gpsimd.partition_broadcast(RB[:], rec[:])
                    h = sb.tile([hx_w, 128], f32, tag="hfin")
                    nc.vector.tensor_tensor(h[:], acc[0:hx_w, :], RB[:],
                                            op=ALU.mult)
                    nc.vector.tensor_scalar(h[:], h[:], bcol, None,
                                            op0=ALU.add)
                    # elu = exp(min(h,0)) - 1 + max(h,0)
                    m = sb.tile([hx_w, 128], f32, tag="melu")
                    nc.vector.tensor_scalar(m[:], h[:], 0.0, None,
                                            op0=ALU.min)
                    nc.scalar.activation(m[:], m[:], AF.Exp)
                    r = sb.tile([hx_w, 128], f32, tag="relu")
                    nc.vector.tensor_scalar(r[:], h[:], 0.0, None,
                                            op0=ALU.max)
                    nc.vector.scalar_tensor_tensor(
                        h[:], m[:], -1.0, r[:], op0=ALU.add, op1=ALU.add)
                    nb_g = w * c.WNB + nbw
                    nc.sync.dma_start(
                        houtT[0:hout_w, nb_g * 128:(nb_g + 1) * 128],
                        h[0:hout_w, :])

        # ================= phases
        build_table(xT, NTILE, Wa1, c.C, c.F, table1, c.EL1)
        build_adown(xTo, c.F, wd1c[:], ad1own)
        edge_pass(1, table1, c.EL1, c.C, ad1own, b1c, h1ownT, c.C)
        build_table(h1ownT, c.NB, Wa2, c.O, c.C, table2own, c.EL2)
        build_adown(h1ownT, c.C, wd2c[:], ad2own)
        if c.NCORES > 1:
            nc.gpsimd.collective_compute(
                "AllGather", mybir.AluOpType.bypass,
                replica_groups=[list(range(c.NCORES))],
                ins=[table2own[:, :].opt()], outs=[table2[:, :].opt()])
            t2 = table2
        else:
            t2 = table2own
        edge_pass(2, t2, c.EL2, c.O, ad2own, b2c, h2ownT, c.O)

    nc.compile()
    return nc


# ===================================================================== glue
def make_in_maps(cfg, inputs, streams):
    c = cfg
    x = np.asarray(inputs["x"], np.float32)
    xp = np.zeros((c.NTOT, c.F), np.float32)
    for ci in range(c.NCORES):
        xp[ci * c.NPCP:ci * c.NPCP + c.NPC] = \
            x[ci * c.NPC:(ci + 1) * c.NPC]
    xT = np.ascontiguousarray(xp.T)

    P8 = np.zeros((8, 64), np.float32)
    for i, k in enumerate(["a_s1", "a_d1", "a_e1", "a_s2", "a_d2", "a_e2",
                           "b1", "b2"]):
        v = np.asarray(inputs[k], np.float32).reshape(-1)
        P8[i, :v.size] = v

    in_maps = []
    for ci in range(c.NCORES):
        xTown = np.ascontiguousarray(
            xT[:, ci * c.NPCP:(ci + 1) * c.NPCP])
        in_maps.append({
            "xT": xT, "xTown": xTown,
            "idx16": streams[ci]["idx16"],
            "dstnb": streams[ci]["dstnb"],
            "ea_ct": streams[ci]["ea_ct"],
            "W1": np.asarray(inputs["W1"], np.float32),
            "W2": np.asarray(inputs["W2"], np.float32),
            "We1": np.asarray(inputs["We1"], np.float32),
            "We2": np.asarray(inputs["We2"], np.float32),
            "P8": P8,
        })
    return in_maps


def assemble_output(cfg, results, mask):
    c = cfg
    cols = []
    for ci in range(c.NCORES):
        cols.append(results[ci]["h2ownT"][:, 0:c.NPC])
    h2 = np.concatenate(cols, axis=1).T  # [N, O]
    return np.ascontiguousarray(h2[np.asarray(mask)])


_CACHE = {}


def run_sharded(cfg, inputs, use_hw=True):
    from concourse import bass_utils
    streams, meta = prepare2(cfg, np.asarray(inputs["edge_index"]),
                             np.asarray(inputs["edge_attr"], np.float32))
    key = (cfg, meta["EP"])
    if key not in _CACHE:
        _CACHE[key] = build_program(cfg, meta)
    nc = _CACHE[key]
    in_maps = make_in_maps(cfg, inputs, streams)
    res = bass_utils.run_bass_kernel_spmd(
        nc, in_maps, core_ids=list(range(cfg.NCORES)))
    return assemble_output(cfg, res.results, inputs["mask"]), res


def kernel(**inputs) -> np.ndarray:
    out, _ = run_sharded(CFG_FULL, inputs, use_hw=True)
    return out

